# revision 1
# baseline (speedup 1.0000x reference)
import sys
for p in ("/opt/trn_rl_repo",):
    if p not in sys.path:
        sys.path.insert(0, p)
"""8-core tensor-parallel GRU recurrence kernel for TRN2 (raw bass).

Design:
 - H=2048, T steps, batch 1. 3H=6144 gate rows sharded 8 ways: core c owns
   rows [256c:256c+256) of each of the r/z/n blocks (768 rows total),
   so it produces h_new[256c:256c+256) each step.
 - W_hh slice lives in SBUF as W^T chunks: wt[p, 768k+n] = W[row_c(n), 128k+p].
 - Per step: 32 accumulating matmuls (lhsT = h k-chunk [128,1], rhs = W^T
   [128,512]/[128,256]) -> gh in PSUM rows [1,512]+[1,256]; ACT/DVE copy to
   SBUF; 6 PE transposes -> [128,*] psum; gates on DVE/ACT in [128,2/4]
   layout; h_new slice [128,2] broadcast SBUF->SBUF to all 8 cores via
   remote_dma_broadcast (slot = own core id, register-offset AP).
 - Double buffering by step parity. All cross-engine sync uses monotonic
   semaphores with per-waiter cumulative-threshold registers (wait_ge(sem,
   reg)); cross-core flow control rides the data dependency chain.
"""
import numpy as np
from concourse import bass, mybir, library_config

H = 2048
NCORES = 8
SLICE = H // NCORES            # 256 h values per core
GROWS = 3 * SLICE              # 768 gate rows per core
KCH = H // 128                 # 16 contraction chunks
F32 = mybir.dt.float32
ADD = mybir.AluOpType.add
MUL = mybir.AluOpType.mult


class Mono:
    """Monotonic semaphore wait: one waiter engine, cumulative register."""

    def __init__(self, nc, eng, name, init=0, sem=None):
        self.eng = eng
        self.sem = sem if sem is not None else nc.alloc_semaphore(name)
        self.reg = eng.alloc_register(f"{name}_cnt")
        eng.reg_mov(self.reg, init)

    def wait(self, n):
        self.eng.reg_add(self.reg, self.reg, n)
        return self.eng.wait_ge(self.sem, self.reg)


def build(T: int, race_check: bool = False):
    # The cross-core WAR edge (remote slot write after local h-buffer reads)
    # is ordered by a two-hop causal chain the race detector cannot trace;
    # all single-core hazards were validated with the detector enabled.
    assert T % 2 == 0
    nc = bass.Bass(
        target_bir_lowering=False, num_devices=NCORES,
        detect_race_conditions=race_check,
    )

    wt_d = nc.dram_tensor("wt", [128, KCH * GROWS], F32, kind="ExternalInput")
    xp_d = nc.dram_tensor("xp", [128, 6 * (T + 4)], F32, kind="ExternalInput")
    bhh_d = nc.dram_tensor("bhhn", [128, 2], F32, kind="ExternalInput")
    out_d = nc.dram_tensor("hout", [128, 2], F32, kind="ExternalOutput")

    wt = nc.alloc_sbuf_tensor("wt_sb", [128, KCH * GROWS], F32)
    xps = [nc.alloc_sbuf_tensor(f"xp{p}", [128, 6], F32) for p in range(2)]
    hbuf = [nc.alloc_sbuf_tensor(f"hbuf{p}", [128, 16], F32) for p in range(2)]
    hnew = [nc.alloc_sbuf_tensor(f"hnew{p}", [128, 2], F32) for p in range(2)]
    ghsb = [nc.alloc_sbuf_tensor(f"ghsb{p}", [1, GROWS], F32) for p in range(2)]
    ones = nc.alloc_sbuf_tensor("ones_sb", [1, 1], F32)
    bhhn = nc.alloc_sbuf_tensor("bhhn_sb", [128, 2], F32)
    tsum = [nc.alloc_sbuf_tensor(f"tsum{p}", [128, 4], F32) for p in range(2)]
    rz = [nc.alloc_sbuf_tensor(f"rz{p}", [128, 4], F32) for p in range(2)]
    hnf = [nc.alloc_sbuf_tensor(f"hnf{p}", [128, 2], F32) for p in range(2)]
    t1 = [nc.alloc_sbuf_tensor(f"t1_{p}", [128, 2], F32) for p in range(2)]
    t2 = [nc.alloc_sbuf_tensor(f"t2_{p}", [128, 2], F32) for p in range(2)]
    t3 = [nc.alloc_sbuf_tensor(f"t3_{p}", [128, 2], F32) for p in range(2)]
    ntv = [nc.alloc_sbuf_tensor(f"nt{p}", [128, 2], F32) for p in range(2)]

    # One full PSUM bank per tensor (P10: no PE-W/DVE-R same-bank overlap).
    pa = [nc.alloc_psum_tensor(f"pa{p}", [128, 512], F32) for p in range(2)]
    pb = [nc.alloc_psum_tensor(f"pb{p}", [128, 512], F32) for p in range(2)]
    prz = [nc.alloc_psum_tensor(f"prz{p}", [128, 512], F32) for p in range(2)]
    pn = [nc.alloc_psum_tensor(f"pn{p}", [128, 512], F32) for p in range(2)]

    te, ve, se, gp, sy = nc.tensor, nc.vector, nc.scalar, nc.gpsimd, nc.sync

    # Monotonic sems, keyed by their single waiter engine.
    rsem = Mono(nc, te, "rsem", init=-16)  # remote arrivals (+2/sender/step)
    cprz = Mono(nc, te, "cprz")       # ACT copied pa -> ghsb rz cols
    cpn = Mono(nc, te, "cpn")         # DVE copied pb -> ghsb n cols
    przf = [Mono(nc, te, f"przf{p}", init=-1) for p in range(2)]
    pnf = [Mono(nc, te, f"pnf{p}", init=-1) for p in range(2)]
    mmrz = Mono(nc, se, "mmrz")       # PE rz matmuls done
    add4 = Mono(nc, se, "add4")       # DVE computed tsum
    t2s = Mono(nc, se, "t2s")         # DVE computed t2 (also last xp read)
    mmn = Mono(nc, ve, "mmn")         # PE n matmuls done
    trrz = Mono(nc, ve, "trrz")       # PE rz transposes done
    trn = Mono(nc, ve, "trn")         # PE n transposes done
    sig = Mono(nc, ve, "sig")         # ACT sigmoid done
    tnh = Mono(nc, ve, "tnh")         # ACT tanh done
    dmax = [Mono(nc, ve, f"dmax{p}") for p in range(2)]   # xp slab loaded
    lsem = [Mono(nc, ve, f"lsem{p}", init=-16) for p in range(2)]  # bcast sent
    hnn = Mono(nc, gp, "hnn")         # DVE wrote hnew
    prep = Mono(nc, gp, "prep")       # bcast descriptors generated
    # sync waits on t2s (the last xp reader's update) with its own registers:
    # parity-0 consumes are increments 1,3,5,..., parity-1 are 2,4,6,...
    cons = [Mono(nc, sy, f"cons{p}", init=(-1 if p == 0 else 0), sem=t2s.sem)
            for p in range(2)]
    init = nc.alloc_semaphore("init")
    done = nc.alloc_semaphore("done")
    odma = nc.alloc_semaphore("odma")
    pdma = nc.alloc_semaphore("pdma")

    # ---------------- prologue ----------------
    sy.dma_start(out=wt[:, :], in_=wt_d[:, :]).then_inc(pdma, 16)
    sy.dma_start(out=bhhn[:, :], in_=bhh_d[:, :]).then_inc(pdma, 16)
    sy.dma_start(out=xps[0][:, :], in_=xp_d[:, 0:6]).then_inc(dmax[0].sem, 16)
    sy.dma_start(out=xps[1][:, :], in_=xp_d[:, 6:12]).then_inc(dmax[1].sem, 16)
    sy.wait_ge(pdma, 32)
    sy.sem_inc(init, 1)

    ve.memset(hbuf[0][:, :], 0.0).then_inc(init, 1)
    ve.memset(hnew[1][:, :], 0.0).then_inc(init, 1)
    ve.memset(ones[:, :], 1.0).then_inc(init, 1)

    gp.load_library(library_config.remote_dma)
    slot_reg = gp.alloc_register("slot_off_reg")
    gp.reg_load(slot_reg, nc.partition_id_tensor[0:1, 0:1])
    gp.reg_alu(slot_reg, slot_reg, 2, op=MUL)
    slot_off = gp.snap(slot_reg, min_val=0, max_val=14)
    # step parity p sends h_new into the *other* parity's h buffer
    bc_out = [hbuf[1][:, bass.ds(slot_off, 2)], hbuf[0][:, bass.ds(slot_off, 2)]]
    rdests = [(0, k) for k in range(NCORES)]

    te.wait_ge(init, 4)
    ve.wait_ge(init, 4)
    gp.wait_ge(init, 4)

    def pe_step(p):
        rsem.wait(16)
        for k in range(KCH):
            i = te.matmul(
                pa[p][0:1, 0:512],
                hbuf[p][:, k : k + 1],
                wt[:, 768 * k : 768 * k + 512],
                start=(k == 0),
                stop=(k == KCH - 1),
            )
        i.then_inc(mmrz.sem, 1)
        for k in range(KCH):
            i = te.matmul(
                pb[p][0:1, 0:256],
                hbuf[p][:, k : k + 1],
                wt[:, 768 * k + 512 : 768 * (k + 1)],
                start=(k == 0),
                stop=(k == KCH - 1),
            )
        i.then_inc(mmn.sem, 1)
        cprz.wait(1)
        przf[p].wait(1)
        for g in range(4):
            i = te.transpose(
                prz[p][:, g : g + 1], ghsb[p][0:1, 128 * g : 128 * (g + 1)],
                ones[0:1, 0:1],
            )
        i.then_inc(trrz.sem, 1)
        cpn.wait(1)
        pnf[p].wait(1)
        for g in (4, 5):
            i = te.transpose(
                pn[p][:, g - 4 : g - 3], ghsb[p][0:1, 128 * g : 128 * (g + 1)],
                ones[0:1, 0:1],
            )
        i.then_inc(trn.sem, 1)

    def act_step(p):
        mmrz.wait(1)
        se.copy(ghsb[p][0:1, 0:512], pa[p][0:1, 0:512]).then_inc(cprz.sem, 1)
        add4.wait(1)
        se.activation(
            rz[p][:, 0:4], tsum[p][:, 0:4], mybir.ActivationFunctionType.Sigmoid
        ).then_inc(sig.sem, 1)
        t2s.wait(1)
        se.activation(
            ntv[p][:, 0:2], t2[p][:, 0:2], mybir.ActivationFunctionType.Tanh
        ).then_inc(tnh.sem, 1)

    def dve_step(p):
        mmn.wait(1)
        ve.tensor_copy(ghsb[p][0:1, 512:768], pb[p][0:1, 0:256]).then_inc(
            cpn.sem, 1
        )
        trrz.wait(1)
        dmax[p].wait(16)
        ve.tensor_add(tsum[p][:, 0:4], prz[p][:, 0:4], xps[p][:, 0:4]).then_inc(
            add4.sem, 1
        )
        ve.sem_inc(przf[p].sem, 1)
        trn.wait(1)
        ve.tensor_add(hnf[p][:, 0:2], pn[p][:, 0:2], bhhn[:, 0:2])
        ve.sem_inc(pnf[p].sem, 1)
        ve.drain()
        sig.wait(1)
        ve.tensor_mul(t1[p][:, 0:2], rz[p][:, 0:2], hnf[p][:, 0:2])
        ve.drain()
        ve.tensor_add(t2[p][:, 0:2], t1[p][:, 0:2], xps[p][:, 4:6]).then_inc(
            t2s.sem, 1
        )
        tnh.wait(1)
        lsem[p].wait(16)
        ve.tensor_sub(t3[p][:, 0:2], hnew[1 - p][:, 0:2], ntv[p][:, 0:2])
        ve.drain()
        ve.tensor_mul(t1[p][:, 0:2], rz[p][:, 2:4], t3[p][:, 0:2])
        ve.drain()
        ve.tensor_add(hnew[p][:, 0:2], ntv[p][:, 0:2], t1[p][:, 0:2]).then_inc(
            hnn.sem, 1
        )

    def pool_step(p):
        gp.remote_dma_broadcast(
            bc_out[p], hnew[p][:, 0:2], remote_sem=rsem.sem, local_sem=lsem[p].sem,
            rdests=rdests,
        ).then_inc(prep.sem, 1)
        hnn.wait(1)
        prep.wait(1)
        gp.trigger_dma(1)

    # sync-engine xp prefetch registers
    offr = [sy.alloc_register("xpoffA"), sy.alloc_register("xpoffB")]
    sy.reg_mov(offr[0], 12)
    sy.reg_mov(offr[1], 18)

    def sync_iter():
        for p in range(2):
            cons[p].wait(2)
            off = sy.snap(offr[p], min_val=12, max_val=6 * T + 6)
            sy.dma_start(out=xps[p][:, :], in_=xp_d[:, bass.ds(off, 6)]).then_inc(
                dmax[p].sem, 16
            )
            sy.reg_alu(offr[p], offr[p], 12, op=ADD)

    with nc.Fori(0, T // 2, 1) as _:
        for p in range(2):
            pe_step(p)
            act_step(p)
            dve_step(p)
            pool_step(p)
        sync_iter()

    # ---------------- epilogue ----------------
    ve.sem_inc(done, 1)
    sy.wait_ge(done, 1)
    sy.dma_start(out=out_d[:, :], in_=hnew[1][:, :]).then_inc(odma, 16)
    sy.wait_ge(odma, 16)
    rsem.wait(16)
    lsem[0].wait(16)
    lsem[1].wait(16)

    from concourse.library_overlay import lower_extended_insts

    lower_extended_insts(nc)
    return nc


# ---------------- host-side packing ----------------

def row_map(c):
    """gh row order for core c: [r rows, z rows, n rows], each 256."""
    base = SLICE * c
    rows = np.concatenate(
        [np.arange(base, base + SLICE),
         H + np.arange(base, base + SLICE),
         2 * H + np.arange(base, base + SLICE)]
    )
    return rows


def pack_inputs(w_hh, x_proj_full, b_hh, T):
    """Per-core input dicts. x_proj_full: (T, 3H) = samples@w_ih.T + b_ih."""
    in_maps = []
    for c in range(NCORES):
        rows = row_map(c)
        wsl = w_hh[rows, :]                       # (768, 2048)
        # wt[p, 768k+n] = wsl[n, 128k+p]
        wt = np.ascontiguousarray(
            wsl.reshape(GROWS, KCH, 128).transpose(2, 1, 0).reshape(128, KCH * GROWS)
        )
        # xp values: for step t, col 6t+g partition p = xval[t, 128g+p]
        xv = x_proj_full[:T, rows].astype(np.float32).copy()  # (T, 768)
        # fold b_hh into the r,z parts (first 512 cols)
        xv[:, :512] += b_hh[rows[:512]]
        xp = np.zeros((128, 6 * (T + 4)), np.float32)
        xp[:, : 6 * T] = (
            xv.reshape(T, 6, 128).transpose(2, 0, 1).reshape(128, 6 * T)
        )
        bn = b_hh[rows[512:]].reshape(2, 128).T.copy()        # [128,2]
        in_maps.append({"wt": wt, "xp": xp, "bhhn": np.ascontiguousarray(bn)})
    return in_maps


def unpack_output(results):
    """results: list of per-core {"hout": [128,2]} -> full h (2048,)."""
    h = np.zeros(H, np.float32)
    for c in range(NCORES):
        sl = results[c]["hout"]                    # [128,2], col j p -> 256c+128j+p
        h[SLICE * c : SLICE * c + 128] = sl[:, 0]
        h[SLICE * c + 128 : SLICE * (c + 1)] = sl[:, 1]
    return h


# ---------------- harness entry point ----------------

T_FULL = 16384
_cache = {}


def _run(inputs, trace=False):
    import os
    samples = np.asarray(inputs["samples"], np.float32)
    w_ih = np.asarray(inputs["w_ih"], np.float32)
    w_hh = np.asarray(inputs["w_hh"], np.float32)
    b_ih = np.asarray(inputs["b_ih"], np.float32)
    b_hh = np.asarray(inputs["b_hh"], np.float32)
    fc_w = np.asarray(inputs["fc_w"], np.float32)
    fc_b = np.asarray(inputs["fc_b"], np.float32)
    T = samples.shape[0]

    x_proj = (samples @ w_ih.T + b_ih).astype(np.float32)
    in_maps = pack_inputs(w_hh, x_proj, b_hh, T)

    if T not in _cache:
        _cache[T] = build(T)
    nc = _cache[T]

    from concourse.bass_utils import run_bass_kernel_spmd

    res = run_bass_kernel_spmd(
        nc, in_maps, core_ids=list(range(NCORES)), trace=trace
    )
    h = unpack_output(res.results)
    out = 1.0 / (1.0 + np.exp(-(h @ fc_w.T + fc_b)))
    return out.reshape(1, 1).astype(np.float32), res


def kernel(**inputs):
    out, _ = _run(inputs, trace=False)
    return out

